# revision 8
# baseline (speedup 1.0000x reference)
"""AttentionConv1d Trainium2 kernel — 8-core batch-parallel SPMD, v2.

Reference semantics (B=8, C=512, T=4096, O=512, K=3):
    out[b,o,t] = sum_{c,k} feature[b,c,t+k-1] * sim[b,(3c+k)//512,t] * weight[o,c,k]
where sim[b,0/1/2,t] = cosine similarity of embedding col t with its left
neighbor / itself / right neighbor.  sim[:,1,:] == 1 (norms >> eps), and for
iid-normal embeddings sim_l/sim_r ~ N(0, 1/C): |sim| <~ 0.2.  The j=0/j=2
conv groups therefore contribute only ~4% of output magnitude, so they run
in fp8 (DoubleRow, 2 chunks per matmul) while the dominant j=1 group stays
bf16.  Per-term fp8 error ~6% x 0.044 contribution => ~2e-3 output error.

Structure per core (one batch element):
  phase 1: j1 conv (4 bf16 matmuls / 128-t tile) -> bf16 stash, with the
      sim reduce interleaved: squares/lag-products per 128-channel tile,
      c-tiles collapsed on DVE (3 adds), ONE ones-matmul partition-reduce
      per quantity (16 N=512 matmuls total vs 64 in v1).
  sims: computed on the partition-replicated reduce rows (no transpose
      matmuls).  sim16 = 16*sim baked on DVE; w_j02 carries x8 host-side;
      the x128 is divided out in the epilogue stt.
  scale: F_j02 (fp8 from host) x sim16 rows -> 4 paired fp8 tiles
      [128, 2, 4112], one pair per DoubleRow matmul.
  phase 2: per 128-t tile: 4 fp8-DR matmuls into one PSUM group, then
      osb = psum/128 + stash (one DVE stt) -> DMA out.
"""
from contextlib import ExitStack

import ml_dtypes
import numpy as np

import concourse.bass as bass
import concourse.tile as tile
from concourse import mybir
from concourse.bass_utils import run_bass_kernel_spmd

F32 = mybir.dt.float32
BF16 = mybir.dt.bfloat16
F8 = mybir.dt.float8e4

B, C, T, O, K = 8, 512, 4096, 512, 3
CP = C // 128  # 4 c-tiles
TQ = T // 128  # 32 t-tiles
NKB = T // 1024  # 4 reduce kilo-blocks

# j-group (3c+k)//512 channel structure: dense 128-channel blocks + 128
# boundary pairs per group (exactly 512 pairs per group, 1536 total).
BIGS = [0, 171, 384]
D_PAIRS = [
    [(c, k) for c in range(128, 170) for k in range(3)] + [(170, 0), (170, 1)],
    [(170, 2)] + [(c, k) for c in range(299, 341) for k in range(3)] + [(341, 0)],
    [(341, 1), (341, 2)] + [(c, k) for c in range(342, 384) for k in range(3)],
]
assert all(len(p) == 128 for p in D_PAIRS)

# fj1 column blocks (start, width) in f_pad coords; blocks overlap 2 cols so
# any 130-col conv window lies inside one block.
F_BLOCKS = [(0, 258), (256, 770), (1024, 1026), (2048, 1026), (3072, 1026)]
F_BLK_Q = [(0, 2), (2, 8), (8, 16), (16, 24), (24, 32)]  # q-tile range per block
_FJ1_STARTS = [0]
for _a, _w in F_BLOCKS:
    _FJ1_STARTS.append(_FJ1_STARTS[-1] + 2 * _w)
FJ1_COLS = _FJ1_STARTS[-1]  # 8212

W8 = 8.0  # host-side scale on w_j02 (keeps fp8 weights in normal range)
S16 = 16.0  # device-side scale on sims (keeps fp8 scaled-F in normal range)
INV_SCALE = 1.0 / (W8 * S16)

F8NP = ml_dtypes.float8_e4m3fn


def host_prep(feature, embedding, weight):
    """Per-core input maps: packed F/E shards + packed weights.

    fj1   [128, 8212] bf16: per block, (BIG1 | bnd1) slices (conv windows)
    fj02  [128, 16400] bf16: big0 | bnd0 | big2 | bnd2, each 4100 cols
    e     [128, 16400] bf16: 4 kilo-blocks x 4 c-tiles x 1025 cols
    wj1   [128, 2048] bf16: 4 j1 chunks x 512 out-channels
    wj02  [128, 8, 512] fp8: 4 DoubleRow pairs x 2 chunks, x8 scaled
    """
    feature = np.ascontiguousarray(np.asarray(feature, dtype=np.float32))
    embedding = np.ascontiguousarray(np.asarray(embedding, dtype=np.float32))
    weight = np.ascontiguousarray(np.asarray(weight, dtype=np.float32))

    f_pad = np.pad(feature, ((0, 0), (0, 0), (1, 1)))  # [B, C, T+2]
    big = {j: f_pad[:, BIGS[j] : BIGS[j] + 128, :] for j in range(3)}
    bnd = {}
    for j in range(3):  # boundary chunks: rows are k-shifted channel copies
        rows = np.stack([f_pad[:, c, k : k + T] for (c, k) in D_PAIRS[j]], axis=1)
        bnd[j] = np.pad(rows, ((0, 0), (0, 0), (0, 2)))  # [B, 128, T+2]

    fj1 = np.concatenate(
        [t[:, :, a : a + w] for (a, w) in F_BLOCKS for t in (big[1], bnd[1])],
        axis=2,
    ).astype(ml_dtypes.bfloat16)  # [B, 128, 8212]

    def pad4100(x):  # [B, 128, T+2] -> [B, 128, 4100]
        return np.pad(x, ((0, 0), (0, 0), (0, 4100 - x.shape[2])))

    fj02 = np.concatenate(
        [pad4100(big[0]), pad4100(bnd[0]), pad4100(big[2]), pad4100(bnd[2])],
        axis=2,
    ).astype(ml_dtypes.bfloat16)  # [B, 128, 16400]

    e_pad = np.pad(embedding, ((0, 0), (0, 0), (1, 0)))  # [B, C, T+1]
    e_packed = np.concatenate(
        [
            e_pad[:, 128 * p : 128 * p + 128, 1024 * kb : 1024 * kb + 1025]
            for kb in range(NKB)
            for p in range(CP)
        ],
        axis=2,
    ).astype(ml_dtypes.bfloat16)  # [B, 128, 16400]

    def w_big(j, k):
        return weight[:, BIGS[j] : BIGS[j] + 128, k].T  # [128, O]

    def w_bnd(j):
        return np.stack([weight[:, c, k] for (c, k) in D_PAIRS[j]], axis=0)

    wj1 = np.concatenate(
        [w_big(1, 0), w_big(1, 1), w_big(1, 2), w_bnd(1)], axis=1
    ).astype(ml_dtypes.bfloat16)  # [128, 2048]

    pairs = [
        (w_big(0, 0), w_big(0, 1)),
        (w_big(0, 2), w_bnd(0)),
        (w_big(2, 0), w_big(2, 1)),
        (w_big(2, 2), w_bnd(2)),
    ]
    wj02 = np.concatenate(
        [np.stack(p, axis=1) for p in pairs], axis=1
    )  # [128, 8, 512]
    wj02 = (wj02 * W8).astype(F8NP)

    in_maps = [
        {
            "feature_j1": np.ascontiguousarray(fj1[b]),
            "feature_j02": np.ascontiguousarray(fj02[b]),
            "embedding": np.ascontiguousarray(e_packed[b]),
            "weight_j1": wj1,
            "weight_j02": wj02,
        }
        for b in range(B)
    ]
    return in_maps


def _fix_sync_waits(nc, limit=1):
    """Split instructions with more sem waits than walrus' TPB encoding allows."""
    counter = 0
    for f in nc.m.functions:
        for bb in f.blocks:
            insts = list(bb.instructions)
            new_insts = []
            changed = False
            for inst in insts:
                si = inst.sync_info
                waits = list(si.on_wait) if si and si.on_wait else []
                if len(waits) > limit:
                    changed = True
                    head, rest = waits[:-limit], waits[-limit:]
                    for i in range(0, len(head), limit):
                        counter += 1
                        nop = mybir.InstNoOp(name=f"I-waitsplit-{counter}")
                        nop.engine = inst.engine
                        nop.sync_info = mybir.SyncInfo(
                            on_wait=head[i : i + limit], on_update=[]
                        )
                        new_insts.append(nop)
                    inst.sync_info = mybir.SyncInfo(
                        on_wait=rest, on_update=list(si.on_update or [])
                    )
                new_insts.append(inst)
            if changed:
                bb.instructions.clear()
                for i in new_insts:
                    bb.add_instruction(i)
    return counter


def build_kernel():
    nc = bass.Bass(target_bir_lowering=False, trn_type="TRN2")
    F1d = nc.declare_dram_parameter("feature_j1", [128, FJ1_COLS], BF16, isOutput=False)
    F2d = nc.declare_dram_parameter("feature_j02", [128, 16400], BF16, isOutput=False)
    Ed = nc.declare_dram_parameter("embedding", [128, 16400], BF16, isOutput=False)
    W1d = nc.declare_dram_parameter("weight_j1", [128, 2048], BF16, isOutput=False)
    W2d = nc.declare_dram_parameter("weight_j02", [128, 8, 512], F8, isOutput=False)
    Od = nc.declare_dram_parameter("out", [T, O], F32, isOutput=True)

    with tile.TileContext(nc) as tc, ExitStack() as ctx:
        body(ctx, tc, F1d, F2d, Ed, W1d, W2d, Od)
    _fix_sync_waits(nc, limit=1)
    return nc


def body(ctx, tc, F1d, F2d, Ed, W1d, W2d, Od):
    nc = tc.nc
    MULT, ADD = mybir.AluOpType.mult, mybir.AluOpType.add

    consts = ctx.enter_context(tc.tile_pool(name="consts", bufs=1))
    fpool = ctx.enter_context(tc.tile_pool(name="fpool", bufs=1))
    f2pool = ctx.enter_context(tc.tile_pool(name="f2pool", bufs=1))
    epool = ctx.enter_context(tc.tile_pool(name="epool", bufs=1))
    wpool = ctx.enter_context(tc.tile_pool(name="wpool", bufs=1))
    sqpool = ctx.enter_context(tc.tile_pool(name="sqpool", bufs=2))
    rowpool = ctx.enter_context(tc.tile_pool(name="rowpool", bufs=1))
    stashpool = ctx.enter_context(tc.tile_pool(name="stashpool", bufs=1))
    outpool = ctx.enter_context(tc.tile_pool(name="outpool", bufs=3))

    # --- constants ---
    ones_t = consts.tile([128, 128], BF16, tag="ones")
    nc.vector.memset(ones_t[:], 1.0)
    e0 = consts.tile([128, 1], BF16, tag="e0")
    nc.vector.memset(e0[:], 0.0)

    # --- DMA priority order: W_j1 + first F_j1 block so the j1 conv starts
    # immediately, E interleaved with the remaining F_j1 blocks for the sim
    # reduce, then the fp8 phase-2 operands.
    wt1 = wpool.tile([128, 2048], BF16, tag="wj1")
    fj1 = fpool.tile([128, FJ1_COLS], BF16, tag="fj1")
    nc.sync.dma_start(wt1[:], W1d[:])
    a, b = _FJ1_STARTS[0], _FJ1_STARTS[1]
    nc.sync.dma_start(fj1[:, a:b], F1d[:, a:b])
    e_kbs = []
    for kb in range(NKB):
        # padded to 4112 cols so the slot can be tag-aliased by an fp8
        # DoubleRow pair tile ([128, 2, 4112] fp8 == [128, 4112] bf16 bytes)
        ekb = epool.tile([128, 4112], BF16, tag=f"ekb{kb}", name=f"ekb{kb}")
        e_kbs.append(ekb[:, 0:4100])
    nc.sync.dma_start(e_kbs[0][:], Ed[:, 0:4100])
    for fb in (1, 2, 3):
        a, b = _FJ1_STARTS[fb], _FJ1_STARTS[fb + 1]
        nc.sync.dma_start(fj1[:, a:b], F1d[:, a:b])
        kb = fb
        nc.sync.dma_start(e_kbs[kb][:], Ed[:, 4100 * kb : 4100 * kb + 4100])
    a, b = _FJ1_STARTS[4], _FJ1_STARTS[5]
    nc.sync.dma_start(fj1[:, a:b], F1d[:, a:b])
    wt2 = wpool.tile([128, 8, 512], F8, tag="wj02")
    nc.sync.dma_start(wt2[:], W2d[:])
    fj02 = f2pool.tile([128, 16400], BF16, tag="fj02")
    for s in range(4):  # big0, bnd0, big2, bnd2
        nc.sync.dma_start(fj02[:, 4100 * s : 4100 * s + 4100],
                          F2d[:, 4100 * s : 4100 * s + 4100])

    # --- sim reduce rows (partition-replicated) ---
    n_row = rowpool.tile([128, T + 2], BF16, tag="n_row")
    dl_row = rowpool.tile([128, T + 2], BF16, tag="dl_row")
    for sb in (n_row, dl_row):
        nc.vector.memset(sb[:, 0:1], 0.0)
        nc.vector.memset(sb[:, T + 1 : T + 2], 0.0)

    def red_kb(kb, s2b, dlb):
        sqs, pls = [], []
        for p in range(CP):
            esl = e_kbs[kb][:, 1025 * p : 1025 * p + 1025]
            sq = sqpool.tile([128, 1024], BF16, tag=f"sq{p}", name=f"sq{kb}_{p}")
            pl = sqpool.tile([128, 1024], BF16, tag=f"pl{p}", name=f"pl{kb}_{p}")
            nc.scalar.square(sq[:], esl[:, 1:1025])  # ACT
            eng = nc.vector if p < 2 else nc.gpsimd
            eng.tensor_mul(pl[:], esl[:, 1:1025], esl[:, 0:1024])
            sqs.append(sq)
            pls.append(pl)
        # collapse 4 c-tiles (in-place adds, sq on DVE / pl on GpSimd), then
        # one ones-matmul partition-reduce per quantity
        for g, eng in ((sqs, nc.vector), (pls, nc.gpsimd)):
            eng.tensor_add(g[0][:], g[0][:], g[1][:])
            eng.tensor_add(g[2][:], g[2][:], g[3][:])
            eng.tensor_add(g[0][:], g[0][:], g[2][:])
        for h in range(2):
            hs = slice(512 * h, 512 * h + 512)
            nc.tensor.matmul(s2b[:, hs], ones_t[:], sqs[0][:, hs], start=True, stop=True)
            nc.tensor.matmul(dlb[:, hs], ones_t[:], pls[0][:, hs], start=True, stop=True)
        # evacuate on ACT: n = sqrt(s2), dl copy
        nc.scalar.sqrt(n_row[:, 1 + 1024 * kb : 1025 + 1024 * kb], s2b[:])
        nc.scalar.copy(dl_row[:, 1 + 1024 * kb : 1025 + 1024 * kb], dlb[:])

    # --- phase 1: j1 conv (sim_c == 1) into bf16 stash, reduce interleaved ---
    def f1_ap(s, off, q):  # s: 0=BIG1, 1=bnd1
        blk = next(i for i, (lo, hi) in enumerate(F_BLK_Q) if lo <= q < hi)
        base = _FJ1_STARTS[blk] + F_BLOCKS[blk][1] * s + off + 128 * q - F_BLOCKS[blk][0]
        return fj1[0:128, base : base + 128]

    J1 = [(0, 0, 0), (0, 1, 1), (0, 2, 2), (1, 0, 3)]  # (s, off, w-chunk)
    red_after = {7: 0, 13: 1, 19: 2, 25: 3}
    stash = []
    with tc.tile_pool(name="cpsum1", bufs=3, space="PSUM") as cpsum1, tc.tile_pool(
        name="redpsum", bufs=1, space="PSUM"
    ) as redpsum:
        # HAM warm-up: dummy 1-col matmuls bridge the first W/F DMA wait so
        # the PE clock gate is at 8/8 when the real conv begins.
        wps = cpsum1.tile([128, O], F32, tag="P", name="warmps")
        for i in range(60):
            nc.tensor.matmul(wps[:, 0:1], ones_t[:], e0[:], start=True, stop=True)
        for q in range(TQ):
            p = cpsum1.tile([128, O], F32, tag="P", name=f"P1_{q}")
            for idx, (s, off, ci) in enumerate(J1):
                nc.tensor.matmul(
                    p[:], f1_ap(s, off, q), wt1[:, 512 * ci : 512 * ci + 512],
                    start=(idx == 0), stop=(idx == 3),
                )
            st = stashpool.tile([128, O], BF16, tag=f"st{q}", name=f"st{q}")
            nc.scalar.copy(st[:], p[:])
            stash.append(st)
            if q in red_after:
                kb = red_after[q]
                s2b = redpsum.tile([128, 1024], F32, tag="s2b", name=f"s2b{kb}")
                dlb = redpsum.tile([128, 1024], F32, tag="dlb", name=f"dlb{kb}")
                red_kb(kb, s2b, dlb)

    # --- sims on replicated rows: prod[v] = n[v]*n[v+1] (v in padded coords);
    # sim16L[u] = 16*dl[u]/prod[u-1..u], sim16R[u] = 16*dl[u+1]/prod[u..u+1]
    prod = rowpool.tile([128, T + 1], BF16, tag="prod")
    nc.gpsimd.tensor_mul(prod[:], n_row[:, 0 : T + 1], n_row[:, 1 : T + 2])
    nc.gpsimd.tensor_scalar_max(prod[:], prod[:], 1e-30)
    with nc.allow_low_precision(reason="sims are ~4% of out; bf16 ample"):
        nc.vector.reciprocal(prod[:], prod[:])
    sim16L = rowpool.tile([128, T], BF16, tag="sim16L")
    nc.vector.scalar_tensor_tensor(
        sim16L[:], dl_row[:, 1 : T + 1], S16, prod[:, 0:T], op0=MULT, op1=MULT
    )
    sim16R = rowpool.tile([128, T], BF16, tag="sim16R")
    nc.vector.scalar_tensor_tensor(
        sim16R[:], dl_row[:, 2 : T + 2], S16, prod[:, 1 : T + 1], op0=MULT, op1=MULT
    )

    # --- scale F_j02 by sim rows into paired fp8 tiles for DoubleRow ---
    fp_pairs = []
    for pi, name in enumerate(("fp00", "fp01", "fp20", "fp21")):
        # tag-alias onto the dead E kilo-block slots (E is consumed by the
        # reduce before any scaled tile is written; same byte size)
        fp = epool.tile([128, 2, 4112], F8, tag=f"ekb{pi}", name=name)
        fp_pairs.append(fp)
    # (pair, half, src-tile index in fj02, col offset, sim row)
    SPECS = [
        (0, 0, 0, 0, sim16L), (0, 1, 0, 1, sim16L),
        (1, 0, 0, 2, sim16L), (1, 1, 1, 0, sim16L),
        (2, 0, 2, 0, sim16R), (2, 1, 2, 1, sim16R),
        (3, 0, 2, 2, sim16R), (3, 1, 3, 0, sim16R),
    ]
    def emit_fsc_batch(c4):
        # batch 0 all-DVE (GpSimd may lag on reduce work); later batches
        # give 2 muls to GpSimd
        a = 1024 * c4
        for i, (pi, half, s, off, srow) in enumerate(SPECS):
            dst = fp_pairs[pi][:, half : half + 1, a : a + 1024].squeeze(1)
            src = fj02[:, 4100 * s + off + a : 4100 * s + off + a + 1024]
            eng = nc.gpsimd if (c4 > 0 and i >= 6) else nc.vector
            eng.tensor_mul(dst, src, srow[:, a : a + 1024])

    # --- phase 2: 4 fp8-DR matmuls per tile + stash add; scale-mul batches
    # interleaved into the emission so the DVE FIFO stays ahead of the PE ---
    DR = mybir.MatmulPerfMode.DoubleRow
    emit_fsc_batch(0)
    fsc_at = {4: 1, 12: 2, 20: 3}
    with tc.tile_pool(name="cpsum2", bufs=3, space="PSUM") as cpsum2:
        for q in range(TQ):
            if q in fsc_at:
                emit_fsc_batch(fsc_at[q])
            p = cpsum2.tile([128, O], F32, tag="P2", name=f"P2_{q}")
            for idx, fp in enumerate(fp_pairs):
                nc.tensor.matmul(
                    p[:], fp[:, :, 128 * q : 128 * q + 128],
                    wt2[:, 2 * idx : 2 * idx + 2, :],
                    start=(idx == 0), stop=(idx == 3), perf_mode=DR,
                )
            osb = outpool.tile([128, O], F32, tag="osb", name=f"osb{q}")
            nc.vector.scalar_tensor_tensor(
                osb[:], p[:], INV_SCALE, stash[q][:], op0=MULT, op1=ADD
            )
            nc.sync.dma_start(Od[128 * q : 128 * q + 128, :], osb[:])


_NC_CACHE = {}


def _get_nc():
    if "nc" not in _NC_CACHE:
        _NC_CACHE["nc"] = build_kernel()
    return _NC_CACHE["nc"]


def kernel(feature, embedding, weight):
    in_maps = host_prep(feature, embedding, weight)
    nc = _get_nc()
    res = run_bass_kernel_spmd(nc, in_maps, core_ids=list(range(B)))
    out = np.stack([res.results[b]["out"].T for b in range(B)])  # [B, O, T]
    return np.ascontiguousarray(out)


# revision 9
# speedup vs baseline: 1.5574x; 1.5574x over previous
"""AttentionConv1d Trainium2 kernel — 8-core batch-parallel SPMD, v2.

Reference semantics (B=8, C=512, T=4096, O=512, K=3):
    out[b,o,t] = sum_{c,k} feature[b,c,t+k-1] * sim[b,(3c+k)//512,t] * weight[o,c,k]
where sim[b,0/1/2,t] = cosine similarity of embedding col t with its left
neighbor / itself / right neighbor.  sim[:,1,:] == 1 (norms >> eps), and for
iid-normal embeddings sim_l/sim_r ~ N(0, 1/C): |sim| <~ 0.2.  The j=0/j=2
conv groups therefore contribute only ~4% of output magnitude, so they run
in fp8 (DoubleRow, 2 chunks per matmul) while the dominant j=1 group stays
bf16.  Per-term fp8 error ~6% x 0.044 contribution => ~2e-3 output error.

Structure per core (one batch element):
  phase 1: j1 conv (4 bf16 matmuls / 128-t tile) -> bf16 stash, with the
      sim reduce interleaved: squares/lag-products per 128-channel tile,
      c-tiles collapsed on DVE (3 adds), ONE ones-matmul partition-reduce
      per quantity (16 N=512 matmuls total vs 64 in v1).
  sims: computed on the partition-replicated reduce rows (no transpose
      matmuls).  sim16 = 16*sim baked on DVE; w_j02 carries x8 host-side;
      the x128 is divided out in the epilogue stt.
  scale: F_j02 (fp8 from host) x sim16 rows -> 4 paired fp8 tiles
      [128, 2, 4112], one pair per DoubleRow matmul.
  phase 2: per 128-t tile: 4 fp8-DR matmuls into one PSUM group, then
      osb = psum/128 + stash (one DVE stt) -> DMA out.
"""
from contextlib import ExitStack

import ml_dtypes
import numpy as np

import concourse.bass as bass
import concourse.tile as tile
from concourse import mybir
from concourse.bass_utils import run_bass_kernel_spmd

F32 = mybir.dt.float32
BF16 = mybir.dt.bfloat16
F8 = mybir.dt.float8e4

B, C, T, O, K = 8, 512, 4096, 512, 3
CP = C // 128  # 4 c-tiles
TQ = T // 128  # 32 t-tiles
NKB = T // 1024  # 4 reduce kilo-blocks

# j-group (3c+k)//512 channel structure: dense 128-channel blocks + 128
# boundary pairs per group (exactly 512 pairs per group, 1536 total).
BIGS = [0, 171, 384]
D_PAIRS = [
    [(c, k) for c in range(128, 170) for k in range(3)] + [(170, 0), (170, 1)],
    [(170, 2)] + [(c, k) for c in range(299, 341) for k in range(3)] + [(341, 0)],
    [(341, 1), (341, 2)] + [(c, k) for c in range(342, 384) for k in range(3)],
]
assert all(len(p) == 128 for p in D_PAIRS)

# fj1 column blocks (start, width) in f_pad coords; blocks overlap 2 cols so
# any 130-col conv window lies inside one block.
F_BLOCKS = [(0, 258), (256, 770), (1024, 1026), (2048, 1026), (3072, 1026)]
F_BLK_Q = [(0, 2), (2, 8), (8, 16), (16, 24), (24, 32)]  # q-tile range per block
_FJ1_STARTS = [0]
for _a, _w in F_BLOCKS:
    _FJ1_STARTS.append(_FJ1_STARTS[-1] + 2 * _w)
FJ1_COLS = _FJ1_STARTS[-1]  # 8212

W8 = 8.0  # host-side scale on w_j02 (keeps fp8 weights in normal range)
S16 = 16.0  # device-side scale on sims (keeps fp8 scaled-F in normal range)
INV_SCALE = 1.0 / (W8 * S16)

F8NP = ml_dtypes.float8_e4m3fn


def host_prep(feature, embedding, weight):
    """Per-core input maps: packed F/E shards + packed weights.

    fj1   [128, 8212] bf16: per block, (BIG1 | bnd1) slices (conv windows)
    fj02  [128, 16400] bf16: big0 | bnd0 | big2 | bnd2, each 4100 cols
    e     [128, 16400] bf16: 4 kilo-blocks x 4 c-tiles x 1025 cols
    wj1   [128, 2048] bf16: 4 j1 chunks x 512 out-channels
    wj02  [128, 8, 512] fp8: 4 DoubleRow pairs x 2 chunks, x8 scaled
    """
    feature = np.ascontiguousarray(np.asarray(feature, dtype=np.float32))
    embedding = np.ascontiguousarray(np.asarray(embedding, dtype=np.float32))
    weight = np.ascontiguousarray(np.asarray(weight, dtype=np.float32))

    f_pad = np.pad(feature, ((0, 0), (0, 0), (1, 1)))  # [B, C, T+2]
    big = {j: f_pad[:, BIGS[j] : BIGS[j] + 128, :] for j in range(3)}
    bnd = {}
    for j in range(3):  # boundary chunks: rows are k-shifted channel copies
        rows = np.stack([f_pad[:, c, k : k + T] for (c, k) in D_PAIRS[j]], axis=1)
        bnd[j] = np.pad(rows, ((0, 0), (0, 0), (0, 2)))  # [B, 128, T+2]

    fj1 = np.concatenate(
        [t[:, :, a : a + w] for (a, w) in F_BLOCKS for t in (big[1], bnd[1])],
        axis=2,
    ).astype(ml_dtypes.bfloat16)  # [B, 128, 8212]

    def pad4100(x):  # [B, 128, T+2] -> [B, 128, 4100]
        return np.pad(x, ((0, 0), (0, 0), (0, 4100 - x.shape[2])))

    fj02 = np.concatenate(
        [pad4100(big[0]), pad4100(bnd[0]), pad4100(big[2]), pad4100(bnd[2])],
        axis=2,
    ).astype(ml_dtypes.bfloat16)  # [B, 128, 16400]

    e_pad = np.pad(embedding, ((0, 0), (0, 0), (1, 0)))  # [B, C, T+1]
    e_packed = np.concatenate(
        [
            e_pad[:, 128 * p : 128 * p + 128, 1024 * kb : 1024 * kb + 1025]
            for kb in range(NKB)
            for p in range(CP)
        ],
        axis=2,
    ).astype(ml_dtypes.bfloat16)  # [B, 128, 16400]

    def w_big(j, k):
        return weight[:, BIGS[j] : BIGS[j] + 128, k].T  # [128, O]

    def w_bnd(j):
        return np.stack([weight[:, c, k] for (c, k) in D_PAIRS[j]], axis=0)

    wj1 = np.concatenate(
        [w_big(1, 0), w_big(1, 1), w_big(1, 2), w_bnd(1)], axis=1
    ).astype(ml_dtypes.bfloat16)  # [128, 2048]

    pairs = [
        (w_big(0, 0), w_big(0, 1)),
        (w_big(0, 2), w_bnd(0)),
        (w_big(2, 0), w_big(2, 1)),
        (w_big(2, 2), w_bnd(2)),
    ]
    wj02 = np.concatenate(
        [np.stack(p, axis=1) for p in pairs], axis=1
    )  # [128, 8, 512]
    wj02 = (wj02 * W8).astype(F8NP)

    in_maps = [
        {
            "feature_j1": np.ascontiguousarray(fj1[b]),
            "feature_j02": np.ascontiguousarray(fj02[b]),
            "embedding": np.ascontiguousarray(e_packed[b]),
            "weight_j1": wj1,
            "weight_j02": wj02,
        }
        for b in range(B)
    ]
    return in_maps


def _fix_sync_waits(nc, limit=1):
    """Split instructions with more sem waits than walrus' TPB encoding allows."""
    counter = 0
    for f in nc.m.functions:
        for bb in f.blocks:
            insts = list(bb.instructions)
            new_insts = []
            changed = False
            for inst in insts:
                si = inst.sync_info
                waits = list(si.on_wait) if si and si.on_wait else []
                if len(waits) > limit:
                    changed = True
                    head, rest = waits[:-limit], waits[-limit:]
                    for i in range(0, len(head), limit):
                        counter += 1
                        nop = mybir.InstNoOp(name=f"I-waitsplit-{counter}")
                        nop.engine = inst.engine
                        nop.sync_info = mybir.SyncInfo(
                            on_wait=head[i : i + limit], on_update=[]
                        )
                        new_insts.append(nop)
                    inst.sync_info = mybir.SyncInfo(
                        on_wait=rest, on_update=list(si.on_update or [])
                    )
                new_insts.append(inst)
            if changed:
                bb.instructions.clear()
                for i in new_insts:
                    bb.add_instruction(i)
    return counter


def build_kernel():
    nc = bass.Bass(target_bir_lowering=False, trn_type="TRN2")
    F1d = nc.declare_dram_parameter("feature_j1", [128, FJ1_COLS], BF16, isOutput=False)
    F2d = nc.declare_dram_parameter("feature_j02", [128, 16400], BF16, isOutput=False)
    Ed = nc.declare_dram_parameter("embedding", [128, 16400], BF16, isOutput=False)
    W1d = nc.declare_dram_parameter("weight_j1", [128, 2048], BF16, isOutput=False)
    W2d = nc.declare_dram_parameter("weight_j02", [128, 8, 512], F8, isOutput=False)
    Od = nc.declare_dram_parameter("out", [T, O], F32, isOutput=True)

    with tile.TileContext(nc) as tc, ExitStack() as ctx:
        body(ctx, tc, F1d, F2d, Ed, W1d, W2d, Od)
    _fix_sync_waits(nc, limit=1)
    return nc


def body(ctx, tc, F1d, F2d, Ed, W1d, W2d, Od):
    nc = tc.nc
    MULT, ADD = mybir.AluOpType.mult, mybir.AluOpType.add

    consts = ctx.enter_context(tc.tile_pool(name="consts", bufs=1))
    fpool = ctx.enter_context(tc.tile_pool(name="fpool", bufs=1))
    f2pool = ctx.enter_context(tc.tile_pool(name="f2pool", bufs=1))
    epool = ctx.enter_context(tc.tile_pool(name="epool", bufs=1))
    wpool = ctx.enter_context(tc.tile_pool(name="wpool", bufs=1))
    sqpool = ctx.enter_context(tc.tile_pool(name="sqpool", bufs=2))
    rowpool = ctx.enter_context(tc.tile_pool(name="rowpool", bufs=1))
    stashpool = ctx.enter_context(tc.tile_pool(name="stashpool", bufs=1))
    outpool = ctx.enter_context(tc.tile_pool(name="outpool", bufs=3))

    # --- constants ---
    ones_t = consts.tile([128, 128], BF16, tag="ones")
    nc.vector.memset(ones_t[:], 1.0)
    e0 = consts.tile([128, 1], BF16, tag="e0")
    nc.vector.memset(e0[:], 0.0)

    # --- DMA priority order: W_j1 + first F_j1 block so the j1 conv starts
    # immediately, E interleaved with the remaining F_j1 blocks for the sim
    # reduce, then the fp8 phase-2 operands.
    wt1 = wpool.tile([128, 2048], BF16, tag="wj1")
    fj1 = fpool.tile([128, FJ1_COLS], BF16, tag="fj1")
    nc.sync.dma_start(wt1[:], W1d[:])
    a, b = _FJ1_STARTS[0], _FJ1_STARTS[1]
    nc.sync.dma_start(fj1[:, a:b], F1d[:, a:b])
    e_kbs = []
    for kb in range(NKB):
        # padded to 4112 cols so the slot can be tag-aliased by an fp8
        # DoubleRow pair tile ([128, 2, 4112] fp8 == [128, 4112] bf16 bytes)
        ekb = epool.tile([128, 4112], BF16, tag=f"ekb{kb}", name=f"ekb{kb}")
        e_kbs.append(ekb[:, 0:4100])
    nc.sync.dma_start(e_kbs[0][:], Ed[:, 0:4100])
    for fb in (1, 2, 3):
        a, b = _FJ1_STARTS[fb], _FJ1_STARTS[fb + 1]
        nc.sync.dma_start(fj1[:, a:b], F1d[:, a:b])
        kb = fb
        nc.sync.dma_start(e_kbs[kb][:], Ed[:, 4100 * kb : 4100 * kb + 4100])
    a, b = _FJ1_STARTS[4], _FJ1_STARTS[5]
    nc.sync.dma_start(fj1[:, a:b], F1d[:, a:b])
    wt2 = wpool.tile([128, 8, 512], F8, tag="wj02")
    nc.sync.dma_start(wt2[:], W2d[:])
    fj02 = f2pool.tile([128, 16400], BF16, tag="fj02")
    for s in range(4):  # big0, bnd0, big2, bnd2
        nc.sync.dma_start(fj02[:, 4100 * s : 4100 * s + 4100],
                          F2d[:, 4100 * s : 4100 * s + 4100])

    # --- sim reduce rows (partition-replicated) ---
    n_row = rowpool.tile([128, T + 2], BF16, tag="n_row")
    dl_row = rowpool.tile([128, T + 2], BF16, tag="dl_row")
    for sb in (n_row, dl_row):
        nc.vector.memset(sb[:, 0:1], 0.0)
        nc.vector.memset(sb[:, T + 1 : T + 2], 0.0)

    def red_kb(kb, s2b, dlb):
        for p in range(CP):
            esl = e_kbs[kb][:, 1025 * p : 1025 * p + 1025]
            sq = sqpool.tile([128, 1024], BF16, tag=f"sq{p}", name=f"sq{kb}_{p}")
            pl = sqpool.tile([128, 1024], BF16, tag=f"pl{p}", name=f"pl{kb}_{p}")
            nc.scalar.square(sq[:], esl[:, 1:1025])  # ACT
            nc.vector.tensor_mul(pl[:], esl[:, 1:1025], esl[:, 0:1024])  # DVE
            # partition-reduce accumulated across the 4 c-tiles on the PE
            for h in range(2):
                hs = slice(512 * h, 512 * h + 512)
                nc.tensor.matmul(s2b[:, hs], ones_t[:], sq[:, hs],
                                 start=(p == 0), stop=(p == CP - 1))
                nc.tensor.matmul(dlb[:, hs], ones_t[:], pl[:, hs],
                                 start=(p == 0), stop=(p == CP - 1))
        # evacuate on ACT: n = sqrt(s2), dl copy
        nc.scalar.sqrt(n_row[:, 1 + 1024 * kb : 1025 + 1024 * kb], s2b[:])
        nc.scalar.copy(dl_row[:, 1 + 1024 * kb : 1025 + 1024 * kb], dlb[:])

    # --- phase 1: j1 conv (sim_c == 1) into bf16 stash, reduce interleaved ---
    def f1_ap(s, off, q):  # s: 0=BIG1, 1=bnd1
        blk = next(i for i, (lo, hi) in enumerate(F_BLK_Q) if lo <= q < hi)
        base = _FJ1_STARTS[blk] + F_BLOCKS[blk][1] * s + off + 128 * q - F_BLOCKS[blk][0]
        return fj1[0:128, base : base + 128]

    J1 = [(0, 0, 0), (0, 1, 1), (0, 2, 2), (1, 0, 3)]  # (s, off, w-chunk)
    red_after = {7: 0, 13: 1, 19: 2, 25: 3}
    stash = []
    with tc.tile_pool(name="cpsum1", bufs=3, space="PSUM") as cpsum1, tc.tile_pool(
        name="redpsum", bufs=1, space="PSUM"
    ) as redpsum:
        # HAM warm-up: dummy 1-col matmuls bridge the first W/F DMA wait so
        # the PE clock gate is at 8/8 when the real conv begins.
        wps = cpsum1.tile([128, O], F32, tag="P", name="warmps")
        for i in range(60):
            nc.tensor.matmul(wps[:, 0:1], ones_t[:], e0[:], start=True, stop=True)
        for q in range(TQ):
            p = cpsum1.tile([128, O], F32, tag="P", name=f"P1_{q}")
            for idx, (s, off, ci) in enumerate(J1):
                nc.tensor.matmul(
                    p[:], f1_ap(s, off, q), wt1[:, 512 * ci : 512 * ci + 512],
                    start=(idx == 0), stop=(idx == 3),
                )
            st = stashpool.tile([128, O], BF16, tag=f"st{q}", name=f"st{q}")
            nc.scalar.copy(st[:], p[:])
            stash.append(st)
            if q in red_after:
                kb = red_after[q]
                s2b = redpsum.tile([128, 1024], F32, tag="s2b", name=f"s2b{kb}")
                dlb = redpsum.tile([128, 1024], F32, tag="dlb", name=f"dlb{kb}")
                red_kb(kb, s2b, dlb)

    # --- sims on replicated rows: prod[v] = n[v]*n[v+1] (v in padded coords);
    # sim16L[u] = 16*dl[u]/prod[u-1..u], sim16R[u] = 16*dl[u+1]/prod[u..u+1]
    prod = rowpool.tile([128, T + 1], BF16, tag="prod")
    nc.vector.tensor_mul(prod[:], n_row[:, 0 : T + 1], n_row[:, 1 : T + 2])
    with nc.allow_low_precision(reason="sims feed the fp8 terms; bf16 ample"):
        nc.vector.reciprocal(prod[:], prod[:])  # edge cols 0/inf -> fixed below
    sim16L = rowpool.tile([128, T], BF16, tag="sim16L")
    nc.vector.scalar_tensor_tensor(
        sim16L[:], dl_row[:, 1 : T + 1], S16, prod[:, 0:T], op0=MULT, op1=MULT
    )
    sim16R = rowpool.tile([128, T], BF16, tag="sim16R")
    nc.vector.scalar_tensor_tensor(
        sim16R[:], dl_row[:, 2 : T + 2], S16, prod[:, 1 : T + 1], op0=MULT, op1=MULT
    )
    nc.vector.memset(sim16L[:, 0:1], 0.0)  # 0*inf NaN at the zero-pad edges
    nc.vector.memset(sim16R[:, T - 1 : T], 0.0)

    # --- scale F_j02 by sim rows into paired fp8 tiles for DoubleRow ---
    fp_pairs = []
    for pi, name in enumerate(("fp00", "fp01", "fp20", "fp21")):
        # tag-alias onto the dead E kilo-block slots (E is consumed by the
        # reduce before any scaled tile is written; same byte size)
        fp = epool.tile([128, 2, 4112], F8, tag=f"ekb{pi}", name=name)
        fp_pairs.append(fp)
    # (pair, half, src-tile index in fj02, col offset, sim row)
    SPECS = [
        (0, 0, 0, 0, sim16L), (0, 1, 0, 1, sim16L),
        (1, 0, 0, 2, sim16L), (1, 1, 1, 0, sim16L),
        (2, 0, 2, 0, sim16R), (2, 1, 2, 1, sim16R),
        (3, 0, 2, 2, sim16R), (3, 1, 3, 0, sim16R),
    ]
    def emit_fsc_batch(c4):
        a = 1024 * c4
        for i, (pi, half, s, off, srow) in enumerate(SPECS):
            dst = fp_pairs[pi][:, half : half + 1, a : a + 1024].squeeze(1)
            src = fj02[:, 4100 * s + off + a : 4100 * s + off + a + 1024]
            nc.vector.tensor_mul(dst, src, srow[:, a : a + 1024])

    # --- phase 2: 4 fp8-DR matmuls per tile + stash add; scale-mul batches
    # interleaved into the emission so the DVE FIFO stays ahead of the PE ---
    DR = mybir.MatmulPerfMode.DoubleRow
    emit_fsc_batch(0)
    fsc_at = {4: 1, 12: 2, 20: 3}
    with tc.tile_pool(name="cpsum2", bufs=3, space="PSUM") as cpsum2:
        for q in range(TQ):
            if q in fsc_at:
                emit_fsc_batch(fsc_at[q])
            p = cpsum2.tile([128, O], F32, tag="P2", name=f"P2_{q}")
            for idx, fp in enumerate(fp_pairs):
                nc.tensor.matmul(
                    p[:], fp[:, :, 128 * q : 128 * q + 128],
                    wt2[:, 2 * idx : 2 * idx + 2, :],
                    start=(idx == 0), stop=(idx == 3), perf_mode=DR,
                )
            osb = outpool.tile([128, O], F32, tag="osb", name=f"osb{q}")
            nc.vector.scalar_tensor_tensor(
                osb[:], p[:], INV_SCALE, stash[q][:], op0=MULT, op1=ADD
            )
            nc.sync.dma_start(Od[128 * q : 128 * q + 128, :], osb[:])


_NC_CACHE = {}


def _get_nc():
    if "nc" not in _NC_CACHE:
        _NC_CACHE["nc"] = build_kernel()
    return _NC_CACHE["nc"]


def kernel(feature, embedding, weight):
    in_maps = host_prep(feature, embedding, weight)
    nc = _get_nc()
    res = run_bass_kernel_spmd(nc, in_maps, core_ids=list(range(B)))
    out = np.stack([res.results[b]["out"].T for b in range(B)])  # [B, O, T]
    return np.ascontiguousarray(out)


# revision 12
# speedup vs baseline: 1.9226x; 1.2345x over previous
"""AttentionConv1d Trainium2 kernel — 8-core batch-parallel SPMD, v5.

Reference semantics (B=8, C=512, T=4096, O=512, K=3):
    out[b,o,t] = sum_{c,k} feature[b,c,t+k-1] * sim[b,(3c+k)//512,t] * weight[o,c,k]
where sim[b,0/1/2,t] = cosine similarity of embedding col t with its left
neighbor / itself / right neighbor.  sim[:,1,:] == 1 (norms >> eps).  For
these inputs sim_l/sim_r carry ~27% of output magnitude, so the j=0/j=2
groups run in fp8 (DoubleRow: 2 chunks per matmul, F and W each quantized
once) while the dominant j=1 group stays bf16: per-term fp8 error ~5% x
0.27 contribution => ~1.3e-2 output error, inside the 2e-2 gate.

Single-pass structure per 128-t output tile q (out_T[t,o] orientation, so
the sims are per-partition scalars applied to PSUM):
    P0 = 2 fp8-DR matmuls (j0 pairs)   -> ACT: tmp = P0 * simL_col[q]
    P2 = 2 fp8-DR matmuls (j2 pairs)   -> DVE: tmp2 = P2 * simR_col[q] + tmp
    P1 = 4 bf16 matmuls (j1) + identity-matmul injecting tmp2
    osb = ACT copy(P1) -> DMA
The first NDEF tiles run before the sims exist: their P0/P2 land in bf16
stashes and the j1 group + epilogue are deferred (v1's trick).

Sim reduce: squares on ACT, lag-products on DVE, c-tiles collapsed by DVE
adds, ONE ones-matmul partition-reduce per quantity per kilo-block, then
the v1 transpose trick (128 N=1 matmuls) turns the replicated rows into
per-partition sim columns (pre-divided by the fp8 weight scale).
"""
from contextlib import ExitStack

import ml_dtypes
import numpy as np

import concourse.bass as bass
import concourse.tile as tile
from concourse import mybir
from concourse.bass_utils import run_bass_kernel_spmd

F32 = mybir.dt.float32
BF16 = mybir.dt.bfloat16
F8 = mybir.dt.float8e4

B, C, T, O, K = 8, 512, 4096, 512, 3
CP = C // 128  # 4 c-tiles
TQ = T // 128  # 32 t-tiles
NKB = T // 1024  # 4 reduce kilo-blocks

# j-group (3c+k)//512 channel structure: dense 128-channel blocks + 128
# boundary pairs per group (exactly 512 pairs per group, 1536 total).
BIGS = [0, 171, 384]
D_PAIRS = [
    [(c, k) for c in range(128, 170) for k in range(3)] + [(170, 0), (170, 1)],
    [(170, 2)] + [(c, k) for c in range(299, 341) for k in range(3)] + [(341, 0)],
    [(341, 1), (341, 2)] + [(c, k) for c in range(342, 384) for k in range(3)],
]
assert all(len(p) == 128 for p in D_PAIRS)

# fj1 column blocks (start, width) in f_pad coords; blocks overlap 2 cols so
# any 130-col conv window lies inside one block.
F_BLOCKS = [(0, 258), (256, 770), (1024, 1026), (2048, 1026), (3072, 1026)]
F_BLK_Q = [(0, 2), (2, 8), (8, 16), (16, 24), (24, 32)]  # q-tile range per block
_FJ1_STARTS = [0]
for _a, _w in F_BLOCKS:
    _FJ1_STARTS.append(_FJ1_STARTS[-1] + 2 * _w)
FJ1_COLS = _FJ1_STARTS[-1]  # 8212

W8 = 8.0  # host-side scale on w_j02 (keeps fp8 weights in normal range)
PAIRW = 4112  # fp8 pair-tile width (pad 4098 -> 16-byte-aligned stride)
NDEF = 16  # tiles whose epilogue is deferred until the sims exist

F8NP = ml_dtypes.float8_e4m3fn


def host_prep(feature, embedding, weight):
    """Per-core input maps.

    fj1   [128, 8212] bf16: per block, (BIG1 | bnd1) slices (conv windows)
    fj02  [128, 8, 4112] fp8: 4 DoubleRow pairs x 2 chunk-halves, aligned
          to output t (offset baked in), unscaled
    e     [128, 16400] bf16: 4 kilo-blocks x 4 c-tiles x 1025 cols
    wj1   [128, 2048] bf16: 4 j1 chunks x 512 out-channels
    wj02  [128, 8, 512] fp8: 4 DoubleRow pairs x 2 chunks, x8 scaled
    """
    feature = np.ascontiguousarray(np.asarray(feature, dtype=np.float32))
    embedding = np.ascontiguousarray(np.asarray(embedding, dtype=np.float32))
    weight = np.ascontiguousarray(np.asarray(weight, dtype=np.float32))

    f_pad = np.pad(feature, ((0, 0), (0, 0), (1, 1)))  # [B, C, T+2]
    big = {j: f_pad[:, BIGS[j] : BIGS[j] + 128, :] for j in range(3)}
    bnd = {}
    for j in range(3):  # boundary chunks: rows are k-shifted channel copies
        rows = np.stack([f_pad[:, c, k : k + T] for (c, k) in D_PAIRS[j]], axis=1)
        bnd[j] = np.pad(rows, ((0, 0), (0, 0), (0, 2)))  # [B, 128, T+2]

    fj1 = np.concatenate(
        [t[:, :, a : a + w] for (a, w) in F_BLOCKS for t in (big[1], bnd[1])],
        axis=2,
    ).astype(ml_dtypes.bfloat16)  # [B, 128, 8212]

    # fp8 pair tiles for DoubleRow: half (pi, i) at col u = f_pad[c, u + off]
    # (off baked in per half, aligned to output t)
    def half(j, off):  # [B, 128, PAIRW]
        src = big[j] if off is not None else bnd[j]
        o = off if off is not None else 0
        h = src[:, :, o : o + T + 2 - o]
        return np.pad(h, ((0, 0), (0, 0), (0, PAIRW - h.shape[2])))

    halves = [
        half(0, 0), half(0, 1), half(0, 2), half(0, None),
        half(2, 0), half(2, 1), half(2, 2), half(2, None),
    ]
    fj02 = np.stack(halves, axis=1).astype(F8NP)  # [B, 8, 128, PAIRW] -> fix axes
    fj02 = np.ascontiguousarray(np.transpose(fj02, (0, 2, 1, 3)))  # [B,128,8,PAIRW]

    e_pad = np.pad(embedding, ((0, 0), (0, 0), (1, 0)))  # [B, C, T+1]
    e_packed = np.concatenate(
        [
            e_pad[:, 128 * p : 128 * p + 128, 1024 * kb : 1024 * kb + 1025]
            for kb in range(NKB)
            for p in range(CP)
        ],
        axis=2,
    ).astype(ml_dtypes.bfloat16)  # [B, 128, 16400]

    def w_big(j, k):
        return weight[:, BIGS[j] : BIGS[j] + 128, k].T  # [128, O]

    def w_bnd(j):
        return np.stack([weight[:, c, k] for (c, k) in D_PAIRS[j]], axis=0)

    wj1 = np.concatenate(
        [w_big(1, 0), w_big(1, 1), w_big(1, 2), w_bnd(1)], axis=1
    ).astype(ml_dtypes.bfloat16)  # [128, 2048]

    pairs = [
        (w_big(0, 0), w_big(0, 1)),
        (w_big(0, 2), w_bnd(0)),
        (w_big(2, 0), w_big(2, 1)),
        (w_big(2, 2), w_bnd(2)),
    ]
    wj02 = np.concatenate(
        [np.stack(p, axis=1) for p in pairs], axis=1
    )  # [128, 8, 512]
    wj02 = (wj02 * W8).astype(F8NP)

    ident = np.eye(128, dtype=ml_dtypes.bfloat16)

    in_maps = [
        {
            "feature_j1": np.ascontiguousarray(fj1[b]),
            "feature_j02": fj02[b],
            "embedding": np.ascontiguousarray(e_packed[b]),
            "weight_j1": wj1,
            "weight_j02": wj02,
            "ident": ident,
        }
        for b in range(B)
    ]
    return in_maps


def _fix_sync_waits(nc, limit=1):
    """Split instructions with more sem waits than walrus' TPB encoding allows."""
    counter = 0
    for f in nc.m.functions:
        for bb in f.blocks:
            insts = list(bb.instructions)
            new_insts = []
            changed = False
            for inst in insts:
                si = inst.sync_info
                waits = list(si.on_wait) if si and si.on_wait else []
                if len(waits) > limit:
                    changed = True
                    head, rest = waits[:-limit], waits[-limit:]
                    for i in range(0, len(head), limit):
                        counter += 1
                        nop = mybir.InstNoOp(name=f"I-waitsplit-{counter}")
                        nop.engine = inst.engine
                        nop.sync_info = mybir.SyncInfo(
                            on_wait=head[i : i + limit], on_update=[]
                        )
                        new_insts.append(nop)
                    inst.sync_info = mybir.SyncInfo(
                        on_wait=rest, on_update=list(si.on_update or [])
                    )
                new_insts.append(inst)
            if changed:
                bb.instructions.clear()
                for i in new_insts:
                    bb.add_instruction(i)
    return counter


def build_kernel():
    nc = bass.Bass(target_bir_lowering=False, trn_type="TRN2")
    F1d = nc.declare_dram_parameter("feature_j1", [128, FJ1_COLS], BF16, isOutput=False)
    F2d = nc.declare_dram_parameter("feature_j02", [128, 8, PAIRW], F8, isOutput=False)
    Ed = nc.declare_dram_parameter("embedding", [128, 16400], BF16, isOutput=False)
    W1d = nc.declare_dram_parameter("weight_j1", [128, 2048], BF16, isOutput=False)
    W2d = nc.declare_dram_parameter("weight_j02", [128, 8, 512], F8, isOutput=False)
    Id = nc.declare_dram_parameter("ident", [128, 128], BF16, isOutput=False)
    Od = nc.declare_dram_parameter("out", [T, O], F32, isOutput=True)
    Dbg = nc.declare_dram_parameter("dbg", [128, 4 * TQ + 2 * TQ], F32, isOutput=True)

    with tile.TileContext(nc) as tc, ExitStack() as ctx:
        body(ctx, tc, F1d, F2d, Ed, W1d, W2d, Id, Od, Dbg)
    _fix_sync_waits(nc, limit=1)
    return nc


def body(ctx, tc, F1d, F2d, Ed, W1d, W2d, Id, Od, Dbg=None):
    nc = tc.nc
    MULT, ADD = mybir.AluOpType.mult, mybir.AluOpType.add
    DR = mybir.MatmulPerfMode.DoubleRow

    consts = ctx.enter_context(tc.tile_pool(name="consts", bufs=1))
    fpool = ctx.enter_context(tc.tile_pool(name="fpool", bufs=1))
    f2pool = ctx.enter_context(tc.tile_pool(name="f2pool", bufs=1))
    epool = ctx.enter_context(tc.tile_pool(name="epool", bufs=1))
    wpool = ctx.enter_context(tc.tile_pool(name="wpool", bufs=1))
    sqpool = ctx.enter_context(tc.tile_pool(name="sqpool", bufs=2))
    rowpool = ctx.enter_context(tc.tile_pool(name="rowpool", bufs=1))
    simpool = ctx.enter_context(tc.tile_pool(name="simpool", bufs=1))
    defpool = ctx.enter_context(tc.tile_pool(name="defpool", bufs=1))
    outpool = ctx.enter_context(tc.tile_pool(name="outpool", bufs=3))

    # --- constants ---
    ones_t = consts.tile([128, 128], BF16, tag="ones")
    nc.vector.memset(ones_t[:], 1.0)
    e0 = consts.tile([128, 1], BF16, tag="e0")
    nc.vector.memset(e0[:], 0.0)
    nc.vector.memset(e0[0:1, :], 1.0)
    ident = consts.tile([128, 128], BF16, tag="ident")

    # --- DMA priority order: W_j1 + first F_j1 block (conv starts ~2us),
    # E interleaved with remaining F_j1 blocks, then the fp8 pair operands.
    wt1 = wpool.tile([128, 2048], BF16, tag="wj1")
    fj1 = fpool.tile([128, FJ1_COLS], BF16, tag="fj1")
    nc.sync.dma_start(wt1[:], W1d[:])
    nc.sync.dma_start(ident[:], Id[:])
    a, b = _FJ1_STARTS[0], _FJ1_STARTS[1]
    nc.sync.dma_start(fj1[:, a:b], F1d[:, a:b])
    wt2 = wpool.tile([128, 8, 512], F8, tag="wj02")
    nc.sync.dma_start(wt2[:], W2d[:])
    fp_pairs = []
    for pi in range(4):
        fp = f2pool.tile([128, 2, PAIRW], F8, tag=f"fp{pi}", name=f"fp{pi}")
        fp_pairs.append(fp)
    nc.sync.dma_start(fp_pairs[0][:], F2d[:, 0:2, :])
    e_kbs = []
    for kb in range(NKB):
        ekb = epool.tile([128, 4100], BF16, tag=f"ekb{kb}", name=f"ekb{kb}")
        e_kbs.append(ekb)
    nc.sync.dma_start(e_kbs[0][:], Ed[:, 0:4100])
    nc.sync.dma_start(fp_pairs[1][:], F2d[:, 2:4, :])
    for kb in (1, 2, 3):
        nc.sync.dma_start(e_kbs[kb][:], Ed[:, 4100 * kb : 4100 * kb + 4100])
        a, b = _FJ1_STARTS[kb], _FJ1_STARTS[kb + 1]
        nc.sync.dma_start(fj1[:, a:b], F1d[:, a:b])
    nc.sync.dma_start(fp_pairs[2][:], F2d[:, 4:6, :])
    a, b = _FJ1_STARTS[4], _FJ1_STARTS[5]
    nc.sync.dma_start(fj1[:, a:b], F1d[:, a:b])
    nc.sync.dma_start(fp_pairs[3][:], F2d[:, 6:8, :])

    # --- sim reduce rows (partition-replicated) ---
    n_row = rowpool.tile([128, T + 2], BF16, tag="n_row")
    dl_row = rowpool.tile([128, T + 2], BF16, tag="dl_row")
    for sb in (n_row, dl_row):
        nc.vector.memset(sb[:, 0:1], 0.0)
        nc.vector.memset(sb[:, T + 1 : T + 2], 0.0)

    def red_kb(kb, s2b, dlb):
        sqs, pls = [], []
        for p in range(CP):
            esl = e_kbs[kb][:, 1025 * p : 1025 * p + 1025]
            sq = sqpool.tile([128, 1024], BF16, tag=f"sq{p}", name=f"sq{kb}_{p}")
            pl = sqpool.tile([128, 1024], BF16, tag=f"pl{p}", name=f"pl{kb}_{p}")
            nc.scalar.square(sq[:], esl[:, 1:1025])  # ACT
            nc.vector.tensor_mul(pl[:], esl[:, 1:1025], esl[:, 0:1024])  # DVE
            sqs.append(sq)
            pls.append(pl)
        for g in (sqs, pls):  # collapse c-tiles on DVE (in-place adds)
            nc.vector.tensor_add(g[0][:], g[0][:], g[1][:])
            nc.vector.tensor_add(g[2][:], g[2][:], g[3][:])
            nc.vector.tensor_add(g[0][:], g[0][:], g[2][:])
        for h in range(2):
            hs = slice(512 * h, 512 * h + 512)
            nc.tensor.matmul(s2b[:, hs], ones_t[:], sqs[0][:, hs], start=True, stop=True)
            nc.tensor.matmul(dlb[:, hs], ones_t[:], pls[0][:, hs], start=True, stop=True)
        nc.scalar.sqrt(n_row[:, 1 + 1024 * kb : 1025 + 1024 * kb], s2b[:])
        nc.scalar.copy(dl_row[:, 1 + 1024 * kb : 1025 + 1024 * kb], dlb[:])

    # --- conv helpers ---
    def f1_ap(s, off, q):  # s: 0=BIG1, 1=bnd1
        blk = next(i for i, (lo, hi) in enumerate(F_BLK_Q) if lo <= q < hi)
        base = _FJ1_STARTS[blk] + F_BLOCKS[blk][1] * s + off + 128 * q - F_BLOCKS[blk][0]
        return fj1[0:128, base : base + 128]

    J1 = [(0, 0, 0), (0, 1, 1), (0, 2, 2), (1, 0, 3)]  # (s, off, w-chunk)

    def mm_j02(psum, side, q):  # side 0 -> pairs 0,1 (j0); side 1 -> pairs 2,3
        for i in range(2):
            pi = 2 * side + i
            nc.tensor.matmul(
                psum[:], fp_pairs[pi][:, :, 128 * q : 128 * q + 128],
                wt2[:, 2 * pi : 2 * pi + 2, :],
                start=(i == 0), stop=(i == 1), perf_mode=DR,
            )

    def mm_j1(psum, q, tmp2):
        for idx, (s, off, ci) in enumerate(J1):
            nc.tensor.matmul(
                psum[:], f1_ap(s, off, q), wt1[:, 512 * ci : 512 * ci + 512],
                start=(idx == 0), stop=False,
            )
        nc.tensor.matmul(psum[:], ident[:], tmp2[:], start=False, stop=True)

    def epilogue(q, p0, p2, p1_and_out):
        # tmp = P0 * simL'[q] (ACT); tmp2 = P2 * simR'[q] + tmp (DVE);
        # then P1 group absorbs tmp2 via the identity matmul; osb = copy(P1)
        tmp = outpool.tile([128, O], BF16, tag="tmp", name=f"tmp{q}")
        nc.scalar.mul(tmp[:], p0, xt_sb[:, q : q + 1])
        tmp2 = outpool.tile([128, O], BF16, tag="tmp2", name=f"tmp2_{q}")
        nc.vector.scalar_tensor_tensor(
            tmp2[:], p2, xt_sb[:, TQ + q : TQ + q + 1], tmp[:],
            op0=MULT, op1=ADD,
        )
        p1_and_out(tmp2)

    def finish(q, p1):
        osb = outpool.tile([128, O], F32, tag="osb", name=f"osb{q}")
        nc.scalar.copy(osb[:], p1[:])
        nc.sync.dma_start(Od[128 * q : 128 * q + 128, :], osb[:])

    # --- phase 1: deferred P0/P2 for the first NDEF tiles (bf16 stash) with
    # the reduce interleaved; sims not yet known.
    a_sbs, c_sbs = {}, {}
    red_after = {2: 0, 6: 1, 10: 2, 14: 3}
    with tc.tile_pool(name="dpsum", bufs=2, space="PSUM") as dpsum, tc.tile_pool(
        name="redpsum", bufs=1, space="PSUM"
    ) as redpsum:
        wps = dpsum.tile([128, O], F32, tag="PD", name="warmps")
        for i in range(80):
            nc.tensor.matmul(wps[:, 0:1], ones_t[:], e0[:], start=True, stop=True)
        for q in range(NDEF):
            pa = dpsum.tile([128, O], F32, tag="PD", name=f"PA{q}")
            mm_j02(pa, 0, q)
            a_sb = defpool.tile([128, O], BF16, tag=f"dA{q}", name=f"dA{q}")
            nc.scalar.copy(a_sb[:], pa[:])
            a_sbs[q] = a_sb
            pc = dpsum.tile([128, O], F32, tag="PD", name=f"PC{q}")
            mm_j02(pc, 1, q)
            c_sb = defpool.tile([128, O], BF16, tag=f"dC{q}", name=f"dC{q}")
            nc.vector.tensor_copy(c_sb[:], pc[:])
            c_sbs[q] = c_sb
            if q in red_after:
                kb = red_after[q]
                s2b = redpsum.tile([128, 1024], F32, tag="s2b", name=f"s2b{kb}")
                dlb = redpsum.tile([128, 1024], F32, tag="dlb", name=f"dlb{kb}")
                red_kb(kb, s2b, dlb)
        # norm-product rows (prodL[t] = n[t]*n[t-1], prodR[t] = n[t]*n[t+1])
        prodL = rowpool.tile([128, T], BF16, tag="prodL", name="prodL")
        nc.vector.tensor_mul(prodL[:], n_row[:, 1 : T + 1], n_row[:, 0:T])
        prodR = rowpool.tile([128, T], BF16, tag="prodR", name="prodR")
        nc.vector.tensor_mul(prodR[:], n_row[:, 1 : T + 1], n_row[:, 2 : T + 2])
        # transpose trick: row value v at col 128q+i -> partition i, col q
        xt_all = dpsum.tile([128, 4 * TQ], F32, tag="PD", name="xt_all")
        variants = [
            ("pL", prodL, 0),
            ("pR", prodR, 0),
            ("dT0", dl_row, 1),
            ("dTp", dl_row, 2),
        ]
        for v, (name, src, off) in enumerate(variants):
            for q in range(TQ):
                nc.tensor.matmul(
                    xt_all[:, 32 * v + q : 32 * v + q + 1],
                    src[:, off + 128 * q : off + 128 * q + 128],
                    e0[:],
                    start=True,
                    stop=True,
                )
        xt_raw = simpool.tile([128, 4 * TQ], F32, tag="xt_raw", name="xt_raw")
        nc.vector.tensor_copy(xt_raw[:], xt_all[:])
    cols = {
        name: xt_raw[:, 32 * v : 32 * v + 32]
        for v, (name, _, _) in enumerate(variants)
    }

    # sims on tiny transposed tiles; [simL' | simR'] where sim' = sim / W8
    # (divides out the x8 on the fp8 weights).
    xt_sb = simpool.tile([128, 2 * TQ], F32, tag="xt_sb", name="xt_sb")
    for i, (px, dx) in enumerate((("pL", "dT0"), ("pR", "dTp"))):
        sl = xt_sb[:, TQ * i : TQ * i + TQ]
        prod = simpool.tile([128, TQ], F32, tag=f"prod_{px}", name=f"prod_{px}")
        nc.vector.tensor_scalar_max(prod[:], cols[px], 1e-30)
        nc.vector.reciprocal(prod[:], prod[:])
        nc.vector.tensor_scalar_mul(prod[:], prod[:], 1.0 / W8)
        nc.vector.tensor_mul(sl, cols[dx], prod[:])

    if Dbg is not None:  # debug: dump raw transposed variants + sim columns
        nc.sync.dma_start(Dbg[:, 0 : 4 * TQ], xt_raw[:])
        nc.sync.dma_start(Dbg[:, 4 * TQ : 6 * TQ], xt_sb[:])

    # --- phase 2: deferred epilogues (j1 + identity inject) interleaved with
    # live tiles.
    convpsum = ctx.enter_context(tc.tile_pool(name="convpsum", bufs=2, space="PSUM"))
    p02psum = ctx.enter_context(tc.tile_pool(name="p02psum", bufs=2, space="PSUM"))

    def deferred_finish(q):
        def p1_and_out(tmp2):
            p1 = convpsum.tile([128, O], F32, tag="P1", name=f"P1d{q}")
            mm_j1(p1, q, tmp2)
            finish(q, p1)
        epilogue(q, a_sbs[q][:], c_sbs[q][:], p1_and_out)

    def live_tile(q):
        pa = p02psum.tile([128, O], F32, tag="PA", name=f"PA{q}")
        mm_j02(pa, 0, q)
        pc = p02psum.tile([128, O], F32, tag="PC", name=f"PC{q}")
        mm_j02(pc, 1, q)

        def p1_and_out(tmp2):
            p1 = convpsum.tile([128, O], F32, tag="P1", name=f"P1_{q}")
            mm_j1(p1, q, tmp2)
            finish(q, p1)
        epilogue(q, pa[:], pc[:], p1_and_out)

    todo = list(range(NDEF))
    for q in range(NDEF, TQ):
        if todo:
            deferred_finish(todo.pop(0))
        live_tile(q)
    for q in todo:
        deferred_finish(q)


_NC_CACHE = {}


def _get_nc():
    if "nc" not in _NC_CACHE:
        _NC_CACHE["nc"] = build_kernel()
    return _NC_CACHE["nc"]


def kernel(feature, embedding, weight):
    in_maps = host_prep(feature, embedding, weight)
    nc = _get_nc()
    res = run_bass_kernel_spmd(nc, in_maps, core_ids=list(range(B)))
    out = np.stack([res.results[b]["out"].T for b in range(B)])  # [B, O, T]
    return np.ascontiguousarray(out)


# revision 14
# speedup vs baseline: 2.0711x; 1.0772x over previous
"""AttentionConv1d Trainium2 kernel — 8-core batch-parallel SPMD, v5.

Reference semantics (B=8, C=512, T=4096, O=512, K=3):
    out[b,o,t] = sum_{c,k} feature[b,c,t+k-1] * sim[b,(3c+k)//512,t] * weight[o,c,k]
where sim[b,0/1/2,t] = cosine similarity of embedding col t with its left
neighbor / itself / right neighbor.  sim[:,1,:] == 1 (norms >> eps).  For
these inputs sim_l/sim_r carry ~27% of output magnitude, so the j=0/j=2
groups run in fp8 (DoubleRow: 2 chunks per matmul, F and W each quantized
once) while the dominant j=1 group stays bf16: per-term fp8 error ~5% x
0.27 contribution => ~1.3e-2 output error, inside the 2e-2 gate.

Single-pass structure per 128-t output tile q (out_T[t,o] orientation, so
the sims are per-partition scalars applied to PSUM):
    P0 = 2 fp8-DR matmuls (j0 pairs)   -> ACT: tmp = P0 * simL_col[q]
    P2 = 2 fp8-DR matmuls (j2 pairs)   -> DVE: tmp2 = P2 * simR_col[q] + tmp
    P1 = 4 bf16 matmuls (j1) + identity-matmul injecting tmp2
    osb = ACT copy(P1) -> DMA
The first NDEF tiles run before the sims exist: their P0/P2 land in bf16
stashes and the j1 group + epilogue are deferred (v1's trick).

Sim reduce: squares on ACT, lag-products on DVE, c-tiles collapsed by DVE
adds, ONE ones-matmul partition-reduce per quantity per kilo-block, then
the v1 transpose trick (128 N=1 matmuls) turns the replicated rows into
per-partition sim columns (pre-divided by the fp8 weight scale).
"""
from contextlib import ExitStack

import ml_dtypes
import numpy as np

import concourse.bass as bass
import concourse.tile as tile
from concourse import mybir
from concourse.bass_utils import run_bass_kernel_spmd

F32 = mybir.dt.float32
BF16 = mybir.dt.bfloat16
F8 = mybir.dt.float8e4

B, C, T, O, K = 8, 512, 4096, 512, 3
CP = C // 128  # 4 c-tiles
TQ = T // 128  # 32 t-tiles
NKB = T // 1024  # 4 reduce kilo-blocks

# j-group (3c+k)//512 channel structure: dense 128-channel blocks + 128
# boundary pairs per group (exactly 512 pairs per group, 1536 total).
BIGS = [0, 171, 384]
D_PAIRS = [
    [(c, k) for c in range(128, 170) for k in range(3)] + [(170, 0), (170, 1)],
    [(170, 2)] + [(c, k) for c in range(299, 341) for k in range(3)] + [(341, 0)],
    [(341, 1), (341, 2)] + [(c, k) for c in range(342, 384) for k in range(3)],
]
assert all(len(p) == 128 for p in D_PAIRS)

# fj1 column blocks (start, width) in f_pad coords; blocks overlap 2 cols so
# any 130-col conv window lies inside one block.
F_BLOCKS = [(0, 258), (256, 770), (1024, 1026), (2048, 1026), (3072, 1026)]
F_BLK_Q = [(0, 2), (2, 8), (8, 16), (16, 24), (24, 32)]  # q-tile range per block
_FJ1_STARTS = [0]
for _a, _w in F_BLOCKS:
    _FJ1_STARTS.append(_FJ1_STARTS[-1] + 2 * _w)
FJ1_COLS = _FJ1_STARTS[-1]  # 8212

W8 = 8.0  # host-side scale on w_j02 (keeps fp8 weights in normal range)
PAIRW = 4112  # fp8 pair-tile width (pad 4098 -> 16-byte-aligned stride)
NDEF = 16  # tiles whose epilogue is deferred until the sims exist

F8NP = ml_dtypes.float8_e4m3fn


def host_prep(feature, embedding, weight):
    """Per-core input maps.

    fj1   [128, 8212] bf16: per block, (BIG1 | bnd1) slices (conv windows)
    fj02  [128, 8, 4112] fp8: 4 DoubleRow pairs x 2 chunk-halves, aligned
          to output t (offset baked in), unscaled
    e     [128, 16400] bf16: 4 kilo-blocks x 4 c-tiles x 1025 cols
    wj1   [128, 2048] bf16: 4 j1 chunks x 512 out-channels
    wj02  [128, 8, 512] fp8: 4 DoubleRow pairs x 2 chunks, x8 scaled
    """
    feature = np.ascontiguousarray(np.asarray(feature, dtype=np.float32))
    embedding = np.ascontiguousarray(np.asarray(embedding, dtype=np.float32))
    weight = np.ascontiguousarray(np.asarray(weight, dtype=np.float32))

    f_pad = np.pad(feature, ((0, 0), (0, 0), (1, 1)))  # [B, C, T+2]
    big = {j: f_pad[:, BIGS[j] : BIGS[j] + 128, :] for j in range(3)}
    bnd = {}
    for j in range(3):  # boundary chunks: rows are k-shifted channel copies
        rows = np.stack([f_pad[:, c, k : k + T] for (c, k) in D_PAIRS[j]], axis=1)
        bnd[j] = np.pad(rows, ((0, 0), (0, 0), (0, 2)))  # [B, 128, T+2]

    fj1 = np.concatenate(
        [t[:, :, a : a + w] for (a, w) in F_BLOCKS for t in (big[1], bnd[1])],
        axis=2,
    ).astype(ml_dtypes.bfloat16)  # [B, 128, 8212]

    # fp8 pair tiles for DoubleRow: half (pi, i) at col u = f_pad[c, u + off]
    # (off baked in per half, aligned to output t)
    def half(j, off):  # [B, 128, PAIRW]
        src = big[j] if off is not None else bnd[j]
        o = off if off is not None else 0
        h = src[:, :, o : o + T + 2 - o]
        return np.pad(h, ((0, 0), (0, 0), (0, PAIRW - h.shape[2])))

    halves = [
        half(0, 0), half(0, 1), half(0, 2), half(0, None),
        half(2, 0), half(2, 1), half(2, 2), half(2, None),
    ]
    fj02 = np.stack(halves, axis=1).astype(F8NP)  # [B, 8, 128, PAIRW] -> fix axes
    fj02 = np.ascontiguousarray(np.transpose(fj02, (0, 2, 1, 3)))  # [B,128,8,PAIRW]

    e_pad = np.pad(embedding, ((0, 0), (0, 0), (1, 0)))  # [B, C, T+1]
    e_packed = np.concatenate(
        [
            e_pad[:, 128 * p : 128 * p + 128, 1024 * kb : 1024 * kb + 1025]
            for kb in range(NKB)
            for p in range(CP)
        ],
        axis=2,
    ).astype(ml_dtypes.bfloat16)  # [B, 128, 16400]

    def w_big(j, k):
        return weight[:, BIGS[j] : BIGS[j] + 128, k].T  # [128, O]

    def w_bnd(j):
        return np.stack([weight[:, c, k] for (c, k) in D_PAIRS[j]], axis=0)

    wj1 = np.concatenate(
        [w_big(1, 0), w_big(1, 1), w_big(1, 2), w_bnd(1)], axis=1
    ).astype(ml_dtypes.bfloat16)  # [128, 2048]

    pairs = [
        (w_big(0, 0), w_big(0, 1)),
        (w_big(0, 2), w_bnd(0)),
        (w_big(2, 0), w_big(2, 1)),
        (w_big(2, 2), w_bnd(2)),
    ]
    wj02 = np.concatenate(
        [np.stack(p, axis=1) for p in pairs], axis=1
    )  # [128, 8, 512]
    wj02 = (wj02 * W8).astype(F8NP)

    ident = np.eye(128, dtype=ml_dtypes.bfloat16)

    in_maps = [
        {
            "feature_j1": np.ascontiguousarray(fj1[b]),
            "feature_j02": fj02[b],
            "embedding": np.ascontiguousarray(e_packed[b]),
            "weight_j1": wj1,
            "weight_j02": wj02,
            "ident": ident,
        }
        for b in range(B)
    ]
    return in_maps


def _fix_sync_waits(nc, limit=1):
    """Split instructions with more sem waits than walrus' TPB encoding allows."""
    counter = 0
    for f in nc.m.functions:
        for bb in f.blocks:
            insts = list(bb.instructions)
            new_insts = []
            changed = False
            for inst in insts:
                si = inst.sync_info
                waits = list(si.on_wait) if si and si.on_wait else []
                if len(waits) > limit:
                    changed = True
                    head, rest = waits[:-limit], waits[-limit:]
                    for i in range(0, len(head), limit):
                        counter += 1
                        nop = mybir.InstNoOp(name=f"I-waitsplit-{counter}")
                        nop.engine = inst.engine
                        nop.sync_info = mybir.SyncInfo(
                            on_wait=head[i : i + limit], on_update=[]
                        )
                        new_insts.append(nop)
                    inst.sync_info = mybir.SyncInfo(
                        on_wait=rest, on_update=list(si.on_update or [])
                    )
                new_insts.append(inst)
            if changed:
                bb.instructions.clear()
                for i in new_insts:
                    bb.add_instruction(i)
    return counter


def build_kernel():
    nc = bass.Bass(target_bir_lowering=False, trn_type="TRN2")
    F1d = nc.declare_dram_parameter("feature_j1", [128, FJ1_COLS], BF16, isOutput=False)
    F2d = nc.declare_dram_parameter("feature_j02", [128, 8, PAIRW], F8, isOutput=False)
    Ed = nc.declare_dram_parameter("embedding", [128, 16400], BF16, isOutput=False)
    W1d = nc.declare_dram_parameter("weight_j1", [128, 2048], BF16, isOutput=False)
    W2d = nc.declare_dram_parameter("weight_j02", [128, 8, 512], F8, isOutput=False)
    Id = nc.declare_dram_parameter("ident", [128, 128], BF16, isOutput=False)
    Od = nc.declare_dram_parameter("out", [T, O], F32, isOutput=True)
    Dbg = nc.declare_dram_parameter("dbg", [128, 4 * TQ + 2 * TQ], F32, isOutput=True)

    with tile.TileContext(nc) as tc, ExitStack() as ctx:
        body(ctx, tc, F1d, F2d, Ed, W1d, W2d, Id, Od, Dbg)
    _fix_sync_waits(nc, limit=1)
    return nc


def body(ctx, tc, F1d, F2d, Ed, W1d, W2d, Id, Od, Dbg=None):
    nc = tc.nc
    MULT, ADD = mybir.AluOpType.mult, mybir.AluOpType.add
    DR = mybir.MatmulPerfMode.DoubleRow

    consts = ctx.enter_context(tc.tile_pool(name="consts", bufs=1))
    fpool = ctx.enter_context(tc.tile_pool(name="fpool", bufs=1))
    f2pool = ctx.enter_context(tc.tile_pool(name="f2pool", bufs=1))
    epool = ctx.enter_context(tc.tile_pool(name="epool", bufs=1))
    wpool = ctx.enter_context(tc.tile_pool(name="wpool", bufs=1))
    sqpool = ctx.enter_context(tc.tile_pool(name="sqpool", bufs=2))
    rowpool = ctx.enter_context(tc.tile_pool(name="rowpool", bufs=1))
    simpool = ctx.enter_context(tc.tile_pool(name="simpool", bufs=1))
    defpool = ctx.enter_context(tc.tile_pool(name="defpool", bufs=1))
    outpool = ctx.enter_context(tc.tile_pool(name="outpool", bufs=3))

    # --- constants ---
    ones_t = consts.tile([128, 128], BF16, tag="ones")
    nc.vector.memset(ones_t[:], 1.0)
    e0 = consts.tile([128, 1], BF16, tag="e0")
    nc.vector.memset(e0[:], 0.0)
    nc.vector.memset(e0[0:1, :], 1.0)
    ident = consts.tile([128, 128], BF16, tag="ident")

    # --- DMA priority order: W_j1 + first F_j1 block (conv starts ~2us),
    # E interleaved with remaining F_j1 blocks, then the fp8 pair operands.
    wt1 = wpool.tile([128, 2048], BF16, tag="wj1")
    fj1 = fpool.tile([128, FJ1_COLS], BF16, tag="fj1")
    wt2 = wpool.tile([128, 8, 512], F8, tag="wj02")
    fp_pairs = []
    for pi in range(4):
        fp = f2pool.tile([128, 2, PAIRW], F8, tag=f"fp{pi}", name=f"fp{pi}")
        fp_pairs.append(fp)
    e_kbs = []
    for kb in range(NKB):
        ekb = epool.tile([128, 4100], BF16, tag=f"ekb{kb}", name=f"ekb{kb}")
        e_kbs.append(ekb)

    def fp_chunk(c):  # fp pair col-chunk c for all 4 pairs
        a0, b0 = 1028 * c, 1028 * c + 1028
        for pi in range(4):
            nc.sync.dma_start(fp_pairs[pi][:, :, a0:b0], F2d[:, 2 * pi : 2 * pi + 2, a0:b0])

    nc.sync.dma_start(wt1[:], W1d[:])
    nc.sync.dma_start(wt2[:], W2d[:])
    nc.sync.dma_start(ident[:], Id[:])
    fp_chunk(0)
    a, b = _FJ1_STARTS[0], _FJ1_STARTS[1]
    nc.sync.dma_start(fj1[:, a:b], F1d[:, a:b])
    nc.sync.dma_start(e_kbs[0][:], Ed[:, 0:4100])
    fp_chunk(1)
    for kb in (1, 2, 3):
        nc.sync.dma_start(e_kbs[kb][:], Ed[:, 4100 * kb : 4100 * kb + 4100])
        a, b = _FJ1_STARTS[kb], _FJ1_STARTS[kb + 1]
        nc.sync.dma_start(fj1[:, a:b], F1d[:, a:b])
        if kb < 3:
            fp_chunk(kb + 1)
    a, b = _FJ1_STARTS[4], _FJ1_STARTS[5]
    nc.sync.dma_start(fj1[:, a:b], F1d[:, a:b])

    # --- sim reduce rows (partition-replicated) ---
    n_row = rowpool.tile([128, T + 2], BF16, tag="n_row")
    dl_row = rowpool.tile([128, T + 2], BF16, tag="dl_row")
    for sb in (n_row, dl_row):
        nc.vector.memset(sb[:, 0:1], 0.0)
        nc.vector.memset(sb[:, T + 1 : T + 2], 0.0)

    def red_kb(kb, s2b, dlb):
        sqs, pls = [], []
        for p in range(CP):
            esl = e_kbs[kb][:, 1025 * p : 1025 * p + 1025]
            sq = sqpool.tile([128, 1024], BF16, tag=f"sq{p}", name=f"sq{kb}_{p}")
            pl = sqpool.tile([128, 1024], BF16, tag=f"pl{p}", name=f"pl{kb}_{p}")
            nc.scalar.square(sq[:], esl[:, 1:1025])  # ACT
            nc.vector.tensor_mul(pl[:], esl[:, 1:1025], esl[:, 0:1024])  # DVE
            sqs.append(sq)
            pls.append(pl)
        for g in (sqs, pls):  # collapse c-tiles on DVE (in-place adds)
            nc.vector.tensor_add(g[0][:], g[0][:], g[1][:])
            nc.vector.tensor_add(g[2][:], g[2][:], g[3][:])
            nc.vector.tensor_add(g[0][:], g[0][:], g[2][:])
        for h in range(2):
            hs = slice(512 * h, 512 * h + 512)
            nc.tensor.matmul(s2b[:, hs], ones_t[:], sqs[0][:, hs], start=True, stop=True)
            nc.tensor.matmul(dlb[:, hs], ones_t[:], pls[0][:, hs], start=True, stop=True)
        nc.scalar.sqrt(n_row[:, 1 + 1024 * kb : 1025 + 1024 * kb], s2b[:])
        nc.scalar.copy(dl_row[:, 1 + 1024 * kb : 1025 + 1024 * kb], dlb[:])

    # --- conv helpers ---
    def f1_ap(s, off, q):  # s: 0=BIG1, 1=bnd1
        blk = next(i for i, (lo, hi) in enumerate(F_BLK_Q) if lo <= q < hi)
        base = _FJ1_STARTS[blk] + F_BLOCKS[blk][1] * s + off + 128 * q - F_BLOCKS[blk][0]
        return fj1[0:128, base : base + 128]

    J1 = [(0, 0, 0), (0, 1, 1), (0, 2, 2), (1, 0, 3)]  # (s, off, w-chunk)

    def mm_j02(psum, side, q):  # side 0 -> pairs 0,1 (j0); side 1 -> pairs 2,3
        for i in range(2):
            pi = 2 * side + i
            nc.tensor.matmul(
                psum[:], fp_pairs[pi][:, :, 128 * q : 128 * q + 128],
                wt2[:, 2 * pi : 2 * pi + 2, :],
                start=(i == 0), stop=(i == 1), perf_mode=DR,
            )

    def mm_j1(psum, q, tmp2):
        for idx, (s, off, ci) in enumerate(J1):
            nc.tensor.matmul(
                psum[:], f1_ap(s, off, q), wt1[:, 512 * ci : 512 * ci + 512],
                start=(idx == 0), stop=False,
            )
        nc.tensor.matmul(psum[:], ident[:], tmp2[:], start=False, stop=True)

    def epilogue(q, p0, p2, p1_and_out):
        # tmp = P0 * simL'[q] (ACT); tmp2 = P2 * simR'[q] + tmp (DVE);
        # then P1 group absorbs tmp2 via the identity matmul; osb = copy(P1)
        tmp = outpool.tile([128, O], BF16, tag="tmp", name=f"tmp{q}")
        nc.scalar.mul(tmp[:], p0, xt_sb[:, q : q + 1])
        tmp2 = outpool.tile([128, O], BF16, tag="tmp2", name=f"tmp2_{q}")
        nc.vector.scalar_tensor_tensor(
            tmp2[:], p2, xt_sb[:, TQ + q : TQ + q + 1], tmp[:],
            op0=MULT, op1=ADD,
        )
        p1_and_out(tmp2)

    def finish(q, p1):
        osb = outpool.tile([128, O], F32, tag="osb", name=f"osb{q}")
        nc.scalar.copy(osb[:], p1[:])
        nc.sync.dma_start(Od[128 * q : 128 * q + 128, :], osb[:])

    # --- phase 1: deferred P0/P2 for the first NDEF tiles (bf16 stash) with
    # the reduce AND the sim row-products/transposes pipelined per kilo-block.
    a_sbs, c_sbs = {}, {}
    red_after = {2: 0, 6: 1, 10: 2, 14: 3}
    prodL = rowpool.tile([128, T], BF16, tag="prodL", name="prodL")
    prodR = rowpool.tile([128, T], BF16, tag="prodR", name="prodR")
    with tc.tile_pool(name="dpsum", bufs=2, space="PSUM") as dpsum, tc.tile_pool(
        name="redpsum", bufs=1, space="PSUM"
    ) as redpsum:
        # transpose trick: row value at col 128q+i -> partition i, col 32v+q
        xt_all = redpsum.tile([128, 4 * TQ], F32, tag="xt", name="xt_all")

        def xt_cols(v, src, off, qlo, qhi):
            for qq in range(qlo, qhi):
                nc.tensor.matmul(
                    xt_all[:, 32 * v + qq : 32 * v + qq + 1],
                    src[:, off + 128 * qq : off + 128 * qq + 128],
                    e0[:], start=True, stop=True,
                )

        def sim_rows_kb(kb):
            # prodL[t] = n[t]*n[t-1] for this kb; prodR[t] = n[t]*n[t+1] lags
            # one kb (needs the next kb's first n column), as do the dTp cols.
            lo, hi = 1024 * kb, 1024 * kb + 1024
            nc.vector.tensor_mul(prodL[:, lo:hi], n_row[:, 1 + lo : 1 + hi],
                                 n_row[:, lo:hi])
            xt_cols(0, prodL, 0, 8 * kb, 8 * kb + 8)
            xt_cols(2, dl_row, 1, 8 * kb, 8 * kb + 8)
            kbs = [kb - 1] if kb > 0 else []
            if kb == NKB - 1:
                kbs.append(kb)
            for kb2 in kbs:
                lo2, hi2 = 1024 * kb2, 1024 * kb2 + 1024
                nc.vector.tensor_mul(prodR[:, lo2:hi2], n_row[:, 1 + lo2 : 1 + hi2],
                                     n_row[:, 2 + lo2 : 2 + hi2])
                xt_cols(1, prodR, 0, 8 * kb2, 8 * kb2 + 8)
                xt_cols(3, dl_row, 2, 8 * kb2, 8 * kb2 + 8)

        wps = dpsum.tile([128, O], F32, tag="PD", name="warmps")
        for i in range(80):
            nc.tensor.matmul(wps[:, 0:1], ones_t[:], e0[:], start=True, stop=True)
        for q in range(NDEF):
            pa = dpsum.tile([128, O], F32, tag="PD", name=f"PA{q}")
            mm_j02(pa, 0, q)
            a_sb = defpool.tile([128, O], BF16, tag=f"dA{q}", name=f"dA{q}")
            nc.scalar.copy(a_sb[:], pa[:])
            a_sbs[q] = a_sb
            pc = dpsum.tile([128, O], F32, tag="PD", name=f"PC{q}")
            mm_j02(pc, 1, q)
            c_sb = defpool.tile([128, O], BF16, tag=f"dC{q}", name=f"dC{q}")
            nc.vector.tensor_copy(c_sb[:], pc[:])
            c_sbs[q] = c_sb
            if q in red_after:
                kb = red_after[q]
                s2b = redpsum.tile([128, 1024], F32, tag="s2b", name=f"s2b{kb}")
                dlb = redpsum.tile([128, 1024], F32, tag="dlb", name=f"dlb{kb}")
                red_kb(kb, s2b, dlb)
                sim_rows_kb(kb)
        xt_raw = simpool.tile([128, 4 * TQ], F32, tag="xt_raw", name="xt_raw")
        nc.vector.tensor_copy(xt_raw[:], xt_all[:])
    cols = {name: xt_raw[:, 32 * v : 32 * v + 32]
            for v, name in enumerate(("pL", "pR", "dT0", "dTp"))}

    # sims on tiny transposed tiles; [simL' | simR'] where sim' = sim / W8
    # (divides out the x8 on the fp8 weights).
    xt_sb = simpool.tile([128, 2 * TQ], F32, tag="xt_sb", name="xt_sb")
    for i, (px, dx) in enumerate((("pL", "dT0"), ("pR", "dTp"))):
        sl = xt_sb[:, TQ * i : TQ * i + TQ]
        prod = simpool.tile([128, TQ], F32, tag=f"prod_{px}", name=f"prod_{px}")
        nc.vector.tensor_scalar_max(prod[:], cols[px], 1e-30)
        nc.vector.reciprocal(prod[:], prod[:])
        nc.vector.scalar_tensor_tensor(sl, cols[dx], 1.0 / W8, prod[:],
                                       op0=MULT, op1=MULT)

    if Dbg is not None:  # debug: dump raw transposed variants + sim columns
        nc.sync.dma_start(Dbg[:, 0 : 4 * TQ], xt_raw[:])
        nc.sync.dma_start(Dbg[:, 4 * TQ : 6 * TQ], xt_sb[:])

    # --- phase 2, software-pipelined: tile i's P0/P2 + sim-weighting runs
    # one step ahead of tile i-1's P1 group, so the identity matmul's tmp2
    # operand is ready well before the PE reaches it.
    convpsum = ctx.enter_context(tc.tile_pool(name="convpsum", bufs=2, space="PSUM"))
    p02psum = ctx.enter_context(tc.tile_pool(name="p02psum", bufs=2, space="PSUM"))

    def front(q, live):
        if live:
            pa = p02psum.tile([128, O], F32, tag="PA", name=f"PA{q}")
            mm_j02(pa, 0, q)
            pc = p02psum.tile([128, O], F32, tag="PC", name=f"PC{q}")
            mm_j02(pc, 1, q)
            p0, p2 = pa[:], pc[:]
        else:
            p0, p2 = a_sbs[q][:], c_sbs[q][:]
        tmp = outpool.tile([128, O], BF16, tag="tmp", name=f"tmp{q}")
        nc.scalar.mul(tmp[:], p0, xt_sb[:, q : q + 1])
        tmp2 = outpool.tile([128, O], BF16, tag="tmp2", name=f"tmp2_{q}")
        nc.vector.scalar_tensor_tensor(
            tmp2[:], p2, xt_sb[:, TQ + q : TQ + q + 1], tmp[:],
            op0=MULT, op1=ADD,
        )
        return tmp2

    def back(q, tmp2):
        p1 = convpsum.tile([128, O], F32, tag="P1", name=f"P1_{q}")
        mm_j1(p1, q, tmp2)
        finish(q, p1)

    seq = []
    todo = list(range(NDEF))
    for q in range(NDEF, TQ):
        if todo:
            seq.append((todo.pop(0), False))
        seq.append((q, True))
    seq.extend((q, False) for q in todo)
    pending = None
    for (q, live) in seq:
        t2 = front(q, live)
        if pending is not None:
            back(*pending)
        pending = (q, t2)
    back(*pending)


_NC_CACHE = {}


def _get_nc():
    if "nc" not in _NC_CACHE:
        _NC_CACHE["nc"] = build_kernel()
    return _NC_CACHE["nc"]


def kernel(feature, embedding, weight):
    in_maps = host_prep(feature, embedding, weight)
    nc = _get_nc()
    res = run_bass_kernel_spmd(nc, in_maps, core_ids=list(range(B)))
    out = np.stack([res.results[b]["out"].T for b in range(B)])  # [B, O, T]
    return np.ascontiguousarray(out)
